# revision 8
# baseline (speedup 1.0000x reference)
"""Per-sample batched matmul: out[b,o,f] = sum_i weights[b,o,i] * x[b,i,f].

Sharding: batch (bs=32) split across 8 NeuronCores, 4 samples each, zero
communication. Per sample the kernel needs W_b transposed into [I, O]
stationary layout; accumulating float32r matmuls stream x_b (f32r = fp32
truncated to ~fp22 in the PE at 4x the fp32 rate; PSUM accumulates fp32).

Engine split (vs. the earlier all-PE version): the steady-state W-tile
transposes are moved OFF the Tensor engine onto the Vector engine's
32x32 stream-transpose (SBUF->SBUF, cross-quadrant routing at nch=32),
so the PE runs a pure back-to-back matmul stream. Scalar (ACT) evicts
matmul PSUM; GpSimd (SWDGE) issues output DMAs. Sample 0 bootstraps with
PE transposes paced by the arriving W DMA (the DVE path is ~4x slower
than PE consumption for a cold first pass).
"""

import sys

try:  # concourse (Bass/Tile) ships in the container, not on default sys.path
    import concourse  # noqa: F401
except ImportError:
    sys.path.insert(0, "/opt/trn_rl_repo")

import numpy as np

BS, IN_SIZE, OUT_SIZE, FEATS = 32, 1024, 1024, 2048
N_CORES = 8
BPC = BS // N_CORES  # samples per core

P = 128
N_FREE = 512  # moving-operand free dim per matmul (1 PSUM bank of fp32)
KO = IN_SIZE // P  # 8 contraction tiles
MO = OUT_SIZE // P  # 8 output-row tiles
NF = FEATS // N_FREE  # 4 output-col chunks

_NC_CACHE = {}


def _build_nc():
    import concourse.mybir as mybir
    import concourse.tile as tile
    from concourse import bacc

    f32 = mybir.dt.float32
    f32r = mybir.dt.float32r

    nc = bacc.Bacc("TRN2", target_bir_lowering=False, debug=False)
    x_d = nc.dram_tensor(
        "x", [BPC, IN_SIZE, FEATS], f32, kind="ExternalInput"
    ).ap()
    w_d = nc.dram_tensor(
        "w", [BPC, OUT_SIZE, IN_SIZE], f32, kind="ExternalInput"
    ).ap()
    o_d = nc.dram_tensor(
        "out", [BPC, OUT_SIZE, FEATS], f32, kind="ExternalOutput"
    ).ap()

    with tile.TileContext(nc) as tc:
        with (
            tc.tile_pool(name="const", bufs=1) as const,
            tc.tile_pool(name="wr_pool", bufs=2) as wr_pool,
            tc.tile_pool(name="wt_pool", bufs=2) as wt_pool,
            tc.tile_pool(name="xn_pool", bufs=3) as xn_pool,
            tc.tile_pool(name="ot_pool", bufs=6) as ot_pool,
            tc.tile_pool(name="st_pool", bufs=2) as st_pool,
            tc.tile_pool(name="psmm", bufs=4, space="PSUM") as psmm_pool,
            tc.tile_pool(name="pstr", bufs=3, space="PSUM") as pstr_pool,
        ):
            eye_d = nc.inline_tensor(np.eye(P, dtype=np.float32), name="eye")
            ident = const.tile([P, P], f32r, name="identr")
            nc.sync.dma_start(ident[:], eye_d.ap().bitcast(f32r))

            # W sample staging, one tile per mo-PAIR (so a DVE transpose
            # depends only on its own pair's DMAs, not the whole sample):
            # wrs[mo//2][p, mo%2, jj, c, s] = W_b[mo*128+p, jj*128+c*32+s].
            # f32r so both the PE transpose-matmul (f32r) and the W DMA
            # satisfy the verifier's f32r-producer rule; the DVE
            # stream-transpose reads it via an fp32 bitcast.
            def new_wr(b):
                return [
                    wr_pool.tile(
                        [P, 2, KO, 4, 32], f32r, tag=f"wr{p}", name=f"wr_{b}_{p}"
                    )
                    for p in range(MO // 2)
                ]

            def load_w_block(b, wrs, mo, ways=2):
                src = w_d[b].rearrange("(mo p) i -> p mo i", p=P)
                wr = wrs[mo // 2]
                dst = wr[:].rearrange("p m jj c s -> p m (jj c s)")
                w = IN_SIZE // ways
                for q in range(ways):
                    nc.sync.dma_start(
                        dst[:, mo % 2, q * w : (q + 1) * w],
                        src[:, mo, q * w : (q + 1) * w].bitcast(f32r),
                    )

            def new_wt(b):
                return wt_pool.tile(
                    [P, KO, MO, P], f32r, tag="wt", name=f"wt_{b}"
                )

            def load_xn(b, n):
                """x chunk [128, KO, 512], split across 2 DMAs."""
                x_r = x_d[b].rearrange("(ko p) f -> p ko f", p=P)
                xn = xn_pool.tile(
                    [P, KO, N_FREE], f32r, tag="xn", name=f"xn_{b}_{n}"
                )
                h = KO // 2
                src = x_r[:, :, n * N_FREE : (n + 1) * N_FREE].bitcast(f32r)
                nc.sync.dma_start(xn[:, :h], src[:, :h])
                nc.sync.dma_start(xn[:, h:], src[:, h:])
                return xn

            def dve_transpose(b, wrs, wt, mo_lo, mo_hi):
                """W row-blocks [mo_lo, mo_hi) -> wt, via the Vector engine.
                16 stream-transpose ops; op (a, c) moves every 32x32 block
                whose i%128 is in [32c, 32c+32) from o-rows 32a+32t+128*mo,
                landing transposed at staging partition group c
                (cross-quadrant write, nch=32). A Vector copy then casts the
                fp32 staging tile into the f32r wt (the BIR verifier requires
                f32r matmul inputs to come from an f32r-rounding producer,
                which StreamTranspose cannot be; same-queue placement keeps
                the cast from head-of-line blocking another engine)."""
                nmo = mo_hi - mo_lo
                wr = wrs[mo_lo // 2]
                m0 = mo_lo % 2
                st = st_pool.tile(
                    [P, KO, 2, P], f32, tag="st", name=f"st_{b}_{mo_lo}"
                )
                for a in range(4):
                    for c in range(4):
                        in_ = wr[
                            32 * a : 32 * (a + 1), m0 : m0 + nmo, :, c, :
                        ].bitcast(f32)
                        out_ = st[
                            32 * c : 32 * (c + 1),
                            :,
                            :nmo,
                            32 * a : 32 * (a + 1),
                        ].rearrange("p ko mo s -> p mo ko s")
                        nc.vector.transpose(out_, in_)
                nc.vector.tensor_copy(
                    out=wt[:, :, mo_lo:mo_hi, :], in_=st[:, :, :nmo, :]
                )

            TG = 4  # transposes packed per PSUM bank

            def pe_transpose_block(b, wrs, wt, mo):
                """Sample-0 bootstrap: one W row-block via 8 PE transposes,
                4 per PSUM bank, evicted by one wide Scalar copy."""
                for g in range(KO // TG):
                    pt = pstr_pool.tile(
                        [P, TG * P], f32r, tag="pt", name=f"pt_{b}_{mo}_{g}"
                    )
                    for c in range(TG):
                        ko = g * TG + c
                        src = wrs[mo // 2][:, mo % 2, ko].rearrange(
                            "p c s -> p (c s)"
                        )
                        nc.tensor.transpose(
                            pt[:, c * P : (c + 1) * P], src, ident[:]
                        )
                    dst = wt[:, g * TG : (g + 1) * TG, mo, :]
                    srcp = pt[:].rearrange("p (c q) -> p c q", c=TG)
                    nc.scalar.copy(dst, srcp)

            def mm_group(b, n, mo, xn, wt, out_splits=1):
                """One [128, 512] output tile: 8 accumulating f32r matmuls,
                Scalar psum eviction, output DMA on GpSimd (SWDGE)."""
                ps = psmm_pool.tile(
                    [P, N_FREE], f32, tag="ps", name=f"ps_{b}_{n}_{mo}"
                )
                for ko in range(KO):
                    nc.tensor.matmul(
                        ps[:],
                        wt[:, ko, mo, :],
                        xn[:, ko, :],
                        start=(ko == 0),
                        stop=(ko == KO - 1),
                    )
                ot = ot_pool.tile(
                    [P, N_FREE], f32, tag="ot", name=f"ot_{b}_{n}_{mo}"
                )
                nc.scalar.copy(ot[:], ps[:])
                dst = o_d[
                    b, mo * P : (mo + 1) * P, n * N_FREE : (n + 1) * N_FREE
                ]
                w = N_FREE // out_splits
                for q in range(out_splits):
                    nc.gpsimd.dma_start(
                        dst[:, q * w : (q + 1) * w], ot[:, q * w : (q + 1) * w]
                    )

            # ---- sample 0: DMA-paced bootstrap ----------------------------
            wr0 = new_wr(0)
            wt0 = new_wt(0)
            # W emission order: mo0/mo1 first (PE transposes start fast),
            # then the first x chunk, then the remaining row-blocks in PE
            # consumption order.
            load_w_block(0, wr0, 0, ways=4)
            load_w_block(0, wr0, 1, ways=4)
            xn_cur = load_xn(0, 0)
            for mo in (2, 3, 4, 5, 6, 7):
                load_w_block(0, wr0, mo, ways=2)
            xn_next = load_xn(0, 1)

            # PE bootstrap: first two row-blocks as soon as W lands, then
            # HAM warm-up dummies to bridge the gap until x arrives (tiny
            # Scalar sink reads keep them live), then n=0 matmul groups
            # interleaved with the remaining row-block transposes.
            pe_transpose_block(0, wr0, wt0, 0)
            pe_transpose_block(0, wr0, wt0, 1)
            warm_sink = const.tile([P, 16], f32r, name="warm_sink")
            for wg in range(4):
                ptw = pstr_pool.tile([P, TG * P], f32r, tag="pt", name=f"ptw_{wg}")
                for c in range(TG):
                    nc.tensor.transpose(
                        ptw[:, c * P : (c + 1) * P], ident[:], ident[:]
                    )
                nc.scalar.copy(warm_sink[:], ptw[:, :16])

            wr1 = new_wr(1)
            wt1 = new_wt(1)
            for mo in range(MO):
                mm_group(0, 0, mo, xn_cur, wt0)
                if mo < 6:
                    pe_transpose_block(0, wr0, wt0, mo + 2)
            # next sample's first W pair goes out as early as the DMA queue
            # allows, so the DVE builds pipeline lead before b1 needs wt1
            load_w_block(1, wr1, 0, ways=2)
            load_w_block(1, wr1, 1, ways=2)
            dve_transpose(1, wr1, wt1, 0, 2)

            # ---- steady pipeline ------------------------------------------
            wt_cur = wt0
            for b in range(BPC):
                start_n = 1 if b == 0 else 0
                last = b == BPC - 1
                wr_nxt = wt_nxt = None
                if not last:
                    if b == 0:
                        wr_nxt, wt_nxt = wr1, wt1
                    else:
                        wr_nxt = new_wr(b + 1)
                        wt_nxt = new_wt(b + 1)

                for n in range(start_n, NF):
                    xn = xn_next
                    # prefetch emission: next x chunk, or next sample's first
                    if n + 1 < NF:
                        xn_next = load_xn(b, n + 1)
                    elif not last:
                        xn_next = load_xn(b + 1, 0)
                    # next sample's W, one mo-pair per n-slot (sample 0 has
                    # three slots: fold pair 3 into the last); each pair is
                    # DVE-transposed as soon as its strips land
                    if not last:
                        plist = {1: [1], 2: [2], 3: [3]}[n] if b == 0 else [n]
                        for pr in plist:
                            load_w_block(b + 1, wr_nxt, 2 * pr, ways=2)
                            load_w_block(b + 1, wr_nxt, 2 * pr + 1, ways=2)
                            dve_transpose(b + 1, wr_nxt, wt_nxt, 2 * pr, 2 * pr + 2)
                    for mo in range(MO):
                        splits = 2 if (last and n == NF - 1 and mo >= MO - 2) else 1
                        mm_group(b, n, mo, xn, wt_cur, out_splits=splits)

                wt_cur = wt_nxt

    nc.compile()
    return nc


def run(x, weights, trace=False):
    """Shard on batch, run SPMD on 8 cores, gather. Returns (out, results)."""
    from concourse.bass_utils import run_bass_kernel_spmd

    key = "nc"
    if key not in _NC_CACHE:
        _NC_CACHE[key] = _build_nc()
    nc = _NC_CACHE[key]

    x = np.ascontiguousarray(np.asarray(x, dtype=np.float32))
    weights = np.ascontiguousarray(np.asarray(weights, dtype=np.float32))
    in_maps = [
        {
            "x": x[c * BPC : (c + 1) * BPC],
            "w": weights[c * BPC : (c + 1) * BPC],
        }
        for c in range(N_CORES)
    ]
    last_err = None
    for attempt in range(3):
        try:
            res = run_bass_kernel_spmd(
                nc, in_maps, core_ids=list(range(N_CORES)), trace=trace
            )
            break
        except Exception as e:  # transient NRT device faults: back off, retry
            last_err = e
            import time as _time

            _time.sleep(5 * (attempt + 1))
    else:
        raise last_err
    out = np.concatenate([res.results[c]["out"] for c in range(N_CORES)], axis=0)
    return out, res


def kernel(x, weights):
    out, _ = run(x, weights, trace=False)
    return out
